# revision 23
# baseline (speedup 1.0000x reference)
"""Trainium2 Bass kernel for nn_Embedding_layer (word emb gather + char CNN).

Computation (per core, data-parallel over batch columns):
  word_emb = W_word[words]                     # [S, B, 300] gather
  char_emb = W_char[chars]                     # [B, S, 16, 50] gather
  y        = conv1d(char_emb, conv_w) + b      # k=3 valid -> [B*S, 200, 14]
  char_feat= max_t(y)                          # [B*S, 200]
  out      = concat(word_emb, char_feat)       # [S, B, 500]

Strategy:
  - 8 cores, each handles 8 of the 64 batch columns (same NEFF, sliced inputs).
  - Word gather: SWDGE indirect DMA of 128 x 1200B rows per word-tile,
    directly into the assembled output SBUF tile.
  - Char path: NO gather. The char vocab is 128 = the partition dim, so the
    embedding lookup + conv collapse into one-hot matmuls on the PE array:
      A_k[v, ch] = sum_c W_char[v, c] * conv_w[ch, c, k]   (k = 0..2)
      y[w, ch, t] = sum_k A_k[char(w, t+k), ch] + b[ch]
    The host pre-permutes char indices to [tile, l*128 + word] order; per
    tile the kernel broadcasts the [1, 2048] index row to 128 partitions
    (gpsimd partition_broadcast), builds a one-hot [128v, 2048] bf16 via
    DVE is_equal against a per-partition iota, and accumulates the three
    taps per output position with PSUM matmuls (lhsT = one-hot column
    block, rhs = A_k). Bias is folded into A_1 via a constant-one row in
    the augmented W_char^T. Maxpool over the 14 positions: DVE strided
    tensor_reduce(max) on PSUM, pairwise max across the two halves.
"""

import numpy as np

import concourse.bass as bass
import concourse.mybir as mybir
import concourse.tile as tile
from concourse import bacc
from concourse.masks import make_identity

DT = mybir.dt

S = 256          # sentence length
B_LOC = 8        # batch columns per core
N_CORES = 8
V_WORD, D_WORD = 50000, 300
V_CHAR, D_CHAR = 128, 50
OUT_CH, KS = 200, 3
L = 16           # word length in chars
T = L - KS + 1   # 14 conv output positions
NT = 16          # word tiles of 128 words (16 sentences x 8 batch) per core
D_OUT = D_WORD + OUT_CH

PSTRIDE = 256    # psum column stride per position (2 positions per 512-f32 bank)


def build_program(reps=1, idx_bcast="pool", do_word=True, do_char=True,
                  fp8=False):
    nc = bacc.Bacc("TRN2", target_bir_lowering=False, debug=False)

    words_g = nc.dram_tensor("words_g", [128, NT], DT.int32, kind="ExternalInput")
    chars_g = nc.dram_tensor("chars_g", [NT, NT * 128], DT.int16, kind="ExternalInput")
    w_word = nc.dram_tensor("W_word", [V_WORD, D_WORD], DT.float32, kind="ExternalInput")
    w_char = nc.dram_tensor("W_char", [V_CHAR, D_CHAR], DT.float32, kind="ExternalInput")
    conv_w = nc.dram_tensor("conv_w", [OUT_CH, D_CHAR, KS], DT.float32, kind="ExternalInput")
    conv_b = nc.dram_tensor("conv_b", [1, OUT_CH], DT.float32, kind="ExternalInput")
    out = nc.dram_tensor("out_loc", [S, B_LOC, D_OUT], DT.float32, kind="ExternalOutput")

    with tile.TileContext(nc) as tc:
        _body(nc, tc, words_g, chars_g, w_word, w_char, conv_w, conv_b, out,
              reps=reps, idx_bcast=idx_bcast, do_word=do_word, do_char=do_char,
              fp8=fp8)

    nc.compile()
    return nc


def _body(nc, tc, words_g, chars_g, w_word, w_char, conv_w, conv_b, out,
          reps=1, idx_bcast="pool", do_word=True, do_char=True, fp8=False):
    with tc.tile_pool(name="const", bufs=1) as cpool:
        # ---- word indices [128, NT]: partition p = si*8 + b, col = g ----
        wi = cpool.tile([128, NT], DT.int32)
        nc.sync.dma_start(out=wi[:], in_=words_g.ap())

        # ---- per-partition iota (value = partition index), f32 for is_equal ----
        iota_i = cpool.tile([128, 1], DT.int32)
        nc.gpsimd.iota(iota_i[:], pattern=[[0, 1]], base=0, channel_multiplier=1)
        iota_p = cpool.tile([128, 1], DT.float32)
        nc.vector.tensor_copy(out=iota_p[:], in_=iota_i[:])

        # ---- char table -> bf16, augmented transpose [51, 128] ----
        wc = cpool.tile([128, D_CHAR], DT.float32)
        nc.sync.dma_start(out=wc[:], in_=w_char.ap())
        wcb = cpool.tile([128, D_CHAR], DT.bfloat16)
        nc.vector.tensor_copy(out=wcb[:], in_=wc[:])

        ident = cpool.tile([128, 128], DT.bfloat16)
        make_identity(nc, ident[:])

        # ---- conv weights: w_k[c, ch] = conv_w[ch, c, k] (bf16), bias row ----
        cw1 = cpool.tile([128, D_CHAR * KS], DT.float32)
        cw2 = cpool.tile([72, D_CHAR * KS], DT.float32)
        cw_flat = conv_w.ap().rearrange("c ch k -> c (ch k)")
        nc.sync.dma_start(out=cw1[:], in_=cw_flat[0:128, :])
        nc.sync.dma_start(out=cw2[:], in_=cw_flat[128:OUT_CH, :])
        cwb1 = cpool.tile([128, D_CHAR * KS], DT.bfloat16)
        cwb2 = cpool.tile([72, D_CHAR * KS], DT.bfloat16)
        nc.vector.tensor_copy(out=cwb1[:], in_=cw1[:])
        nc.vector.tensor_copy(out=cwb2[:], in_=cw2[:])

        # w_k live at partitions 0-49 (+ bias row 50 for k=1)
        wk = [cpool.tile([51, OUT_CH], DT.bfloat16, name=f"wk{k}")
              for k in range(KS)]
        wct = cpool.tile([51, 128], DT.bfloat16)   # augmented W_char^T
        ak = [cpool.tile([128, OUT_CH], DT.bfloat16, name=f"ak{k}")
              for k in range(KS)]

        with tc.tile_pool(name="psum_pre", bufs=2, space="PSUM") as ppre:
            for k in range(KS):
                for c0, cn, src in ((0, 128, cwb1), (128, 72, cwb2)):
                    pt = ppre.tile([128, 128], DT.bfloat16, tag="pt")
                    tin = src[:].rearrange("c (ch k) -> c k ch", k=KS)[:, k, :]
                    nc.tensor.transpose(
                        out=pt[0:D_CHAR, 0:cn],
                        in_=tin,
                        identity=ident[0:cn, 0:cn],
                    )
                    nc.vector.tensor_copy(
                        out=wk[k][0:D_CHAR, c0:c0 + cn],
                        in_=pt[0:D_CHAR, 0:cn],
                    )
            # bias row: cast f32->bf16 during DMA (SWDGE)
            nc.gpsimd.dma_start(out=wk[1][D_CHAR:D_CHAR + 1, :], in_=conv_b.ap())

            # augmented W_char^T: rows 0-49 = W_char^T, row 50 = 1.0
            ptc = ppre.tile([128, 128], DT.bfloat16, tag="pt")
            nc.tensor.transpose(out=ptc[0:D_CHAR, :], in_=wcb[:],
                                identity=ident[:])
            nc.gpsimd.memset(wct[:], 1.0)  # row 50 stays 1.0 (bias row)
            nc.vector.tensor_copy(out=wct[0:D_CHAR, :], in_=ptc[0:D_CHAR, :])

            # A_k[v, ch] = sum_c wct[c, v] * wk[k][c, ch]  (+ bias via row 50)
            # DoubleRow rhs halves must be 16-byte multiples: pad 200 -> 208.
            AKW = 208
            ak01 = cpool.tile([128, 2 * AKW], DT.float8e4)
            if fp8:
                nc.gpsimd.memset(ak01[:], 0.0)
            for k in range(KS):
                kk = D_CHAR + 1 if k == 1 else D_CHAR
                pa = ppre.tile([128, OUT_CH], DT.float32, tag="pa")
                nc.tensor.matmul(out=pa[:], lhsT=wct[0:kk, :],
                                 rhs=wk[k][0:kk, :], start=True, stop=True)
                nc.vector.tensor_copy(out=ak[k][:], in_=pa[:])
                if fp8 and k < 2:
                    # fp8 copy of A_0|A_1 for the DoubleRow tap-pair matmul
                    nc.vector.tensor_copy(
                        out=ak01[:, k * AKW:k * AKW + OUT_CH], in_=pa[:])

        out_view = out.ap().rearrange("(g si) b c -> g si b c", si=16)

        with (
            tc.tile_pool(name="idx", bufs=3) as ipool,
            tc.tile_pool(name="oh", bufs=3) as hpool,
            tc.tile_pool(name="outp", bufs=4) as opool,
            tc.tile_pool(name="mx", bufs=6) as mpool,
            tc.tile_pool(name="psum", bufs=2, space="PSUM") as ppool,
        ):
            import contextlib
            rep_ctx = (tc.For_i(0, reps, 1) if reps > 1
                       else contextlib.nullcontext())
            with rep_ctx:
                _main_loop(nc, tc, ipool, hpool, opool, mpool, ppool,
                           chars_g, wi, w_word, iota_p, ak, ak01, out_view,
                           idx_bcast, do_word, do_char, fp8)


def _main_loop(nc, tc, ipool, hpool, opool, mpool, ppool,
               chars_g, wi, w_word, iota_p, ak, ak01, out_view, idx_bcast,
               do_word=True, do_char=True, fp8=True):
    for g in range(NT):
        # ---- word gather straight into the output tile (SWDGE) ----
        otile = opool.tile([128, D_OUT], DT.float32, tag="otile")
        ct = otile[:, D_WORD:D_OUT]
        if do_word:
            nc.gpsimd.indirect_dma_start(
                out=otile[:, 0:D_WORD],
                out_offset=None,
                in_=w_word.ap(),
                in_offset=bass.IndirectOffsetOnAxis(ap=wi[:, g:g + 1], axis=0),
            )
        else:
            nc.vector.memset(otile[:, 0:D_WORD], 0.0)

        if not do_char:
            nc.vector.memset(ct, 0.0)
            nc.sync.dma_start(out=out_view[g], in_=otile[:])
            continue

        # ---- char index row -> broadcast -> one-hot [128v, 2048] ----
        cidx = ipool.tile([1, NT * 128], DT.int16, tag="cidx")
        nc.scalar.dma_start(out=cidx[:], in_=chars_g.ap()[g:g + 1, :])
        idxb = ipool.tile([128, NT * 128], DT.int16, tag="idxb")
        if idx_bcast == "pool":
            nc.gpsimd.partition_broadcast(idxb[:], cidx[0:1, :])
        else:  # doubling DMA fallback
            nc.sync.dma_start(out=idxb[0:1, :], in_=cidx[0:1, :])
            for n in (1, 2, 4, 8, 16, 32, 64):
                nc.sync.dma_start(out=idxb[n:2 * n, :], in_=idxb[0:n, :])
        if fp8:
            # fp8 one-hot (cols 0..1919) for the DoubleRow tap-0/1 pair,
            # bf16 one-hot (cols 256..2047) for the bf16 tap-2 matmul.
            oh8 = hpool.tile([128, 15 * 128], DT.float8e4, tag="oh8")
            nc.vector.tensor_scalar(
                out=oh8[:], in0=idxb[:, 0:15 * 128], scalar1=iota_p[:],
                scalar2=None, op0=mybir.AluOpType.is_equal)
            ohb = hpool.tile([128, 14 * 128], DT.bfloat16, tag="ohb")
            nc.gpsimd.tensor_scalar(
                out=ohb[:], in0=idxb[:, 2 * 128:], scalar1=iota_p[:],
                scalar2=None, op0=mybir.AluOpType.is_equal)
        else:
            oh = hpool.tile([128, NT * 128], DT.bfloat16, tag="oh")
            nc.vector.tensor_scalar(
                out=oh[:], in0=idxb[:], scalar1=iota_p[:], scalar2=None,
                op0=mybir.AluOpType.is_equal)

        # ---- conv: per position t, PSUM-accumulating tap matmuls ----
        ms = []
        for h in range(2):
            ph = ppool.tile([128, 7 * PSTRIDE], DT.float32, tag="ph")
            for tt in range(7):
                t = h * 7 + tt
                po = ph[:, tt * PSTRIDE: tt * PSTRIDE + OUT_CH]
                if fp8:
                    # taps 0+1 in one fp8 DoubleRow matmul, tap 2 in bf16
                    po8 = ph[:, tt * PSTRIDE: tt * PSTRIDE + 208]
                    nc.tensor.matmul(
                        out=po8,
                        lhsT=oh8[:, t * 128:(t + 2) * 128].rearrange(
                            "p (two f) -> p two f", two=2),
                        rhs=ak01[:].rearrange("p (two f) -> p two f", two=2),
                        start=True, stop=False,
                        perf_mode=mybir.MatmulPerfMode.DoubleRow,
                    )
                    nc.tensor.matmul(
                        out=po,
                        lhsT=ohb[:, t * 128:(t + 1) * 128],
                        rhs=ak[2][:],
                        start=False, stop=True,
                    )
                else:
                    for k in range(KS):
                        nc.tensor.matmul(
                            out=po,
                            lhsT=oh[:, (t + k) * 128:(t + k + 1) * 128],
                            rhs=ak[k][:],
                            start=(k == 0), stop=(k == KS - 1),
                        )
            mh = mpool.tile([128, OUT_CH], DT.float32, tag="mh")
            red_in = ph[:].rearrange("p (t c) -> p c t", t=7)[:, 0:OUT_CH, :]
            nc.vector.tensor_reduce(
                out=mh[:], in_=red_in,
                axis=mybir.AxisListType.X, op=mybir.AluOpType.max,
            )
            ms.append(mh)
        nc.vector.tensor_tensor(
            out=ct, in0=ms[0][:], in1=ms[1][:], op=mybir.AluOpType.max)
        nc.sync.dma_start(out=out_view[g], in_=otile[:])


_CACHE = {}


def _get_program():
    if "nc" not in _CACHE:
        _CACHE["nc"] = build_program()
    return _CACHE["nc"]


def _in_maps(words, chars, W_word, W_char, conv_w, conv_b):
    words = np.asarray(words, dtype=np.int32)
    chars = np.asarray(chars, dtype=np.int32)
    W_word = np.ascontiguousarray(np.asarray(W_word, dtype=np.float32))
    W_char = np.ascontiguousarray(np.asarray(W_char, dtype=np.float32))
    conv_w = np.ascontiguousarray(np.asarray(conv_w, dtype=np.float32))
    conv_b = np.ascontiguousarray(
        np.asarray(conv_b, dtype=np.float32).reshape(1, OUT_CH))

    # words [S=256, 64] -> per core: wi[p = si*8+b, g] (tile g = s//16)
    w4 = words.reshape(NT, 16, N_CORES, B_LOC)          # [g, si, core, b]
    w_all = np.ascontiguousarray(
        w4.transpose(2, 1, 3, 0).reshape(N_CORES, 128, NT))

    # chars [64, S, L] -> per core: cidx[g, l*128 + si*8 + b] (int16)
    c5 = chars.reshape(N_CORES, B_LOC, NT, 16, L)       # [core, b, g, si, l]
    c_all = np.ascontiguousarray(
        c5.transpose(0, 2, 4, 3, 1).reshape(N_CORES, NT, NT * 128)
    ).astype(np.int16)

    maps = []
    for i in range(N_CORES):
        maps.append({
            "words_g": w_all[i],
            "chars_g": c_all[i],
            "W_word": W_word,
            "W_char": W_char,
            "conv_w": conv_w,
            "conv_b": conv_b,
        })
    return maps


def kernel(words, chars, W_word, W_char, conv_w, conv_b, trace=False):
    from concourse import bass_utils
    nc = _get_program()
    maps = _in_maps(words, chars, W_word, W_char, conv_w, conv_b)
    res = bass_utils.run_bass_kernel_spmd(
        nc, maps, core_ids=list(range(N_CORES)), trace=trace)
    full = np.concatenate([r["out_loc"] for r in res.results], axis=1)
    if trace:
        return full, res
    return full
